# revision 32
# baseline (speedup 1.0000x reference)
"""Trainium2 Bass kernel for EnhancedGraphAttentionLayer (B=1, N=1024, D=64).

Sharding: destination-node rows split across 8 cores (128 rows each), fully
replicated h; no collectives.

Per core, rows are processed in PAIRS (i0=2k, i1=2k+1) stacked on the 128
SBUF partitions, which halves both PE column-streaming and vector-engine
instruction count vs. per-row processing.

Math (alpha=0.2 LeakyReLU at both nonlinearities):
  relu_s_ij = relu(ej_j + ei_i + b_e)          # s1, per-dim d
  pre_ij    = 0.8*A_e^T relu_s_ij + V_j + u_i  # M1 (paired matmul) + V,u
  e_ij      = w2^T lrelu(pre_ij)  (+ row consts dropped: softmax-invariant)
with host-precomputed
  V_j = (W A_j + 0.2 E_j A_e)^T h_j,   u_i = (W A_i)^T h_i
        + 0.2 A_e^T (E_i^T h_i + b_e) + b1.

Two equivalent per-pair forms (statically scheduled to balance engines):
  Y-form:  pre = (psum + u) + V  [STT]; y = 0.2*pre; z = max(pre, y)  -> M2a(z)
  Z-form:  relu(pre) = max(psum+u, -V) + V:
           z' = max(psum + u, -V)  [one STT]  -> M2a'(0.8*w2 on z')
           + M2b(0.16*A_e w2 on relu_s) + per-j const c_j = w2^T V_j
           (c_j added during the e bank copy-out, rows masked per-form).
Scores are shifted +4096 at PSUM->SBUF copy so `e*adj` masking underflows
exactly like the reference's -1e9 masked_fill under softmax.
"""
import sys
import numpy as np

if "/opt/trn_rl_repo" not in sys.path:
    sys.path.insert(0, "/opt/trn_rl_repo")

import ml_dtypes
import concourse.bass as bass
import concourse.bacc as bacc
import concourse.mybir as mybir
import concourse.tile as tile
from concourse.bass_utils import run_bass_kernel_spmd

F32 = mybir.dt.float32
BF16 = mybir.dt.bfloat16
AF = mybir.ActivationFunctionType
ALU = mybir.AluOpType
AX = mybir.AxisListType

N = 1024
D = 64
NCORES = 8
R = N // NCORES          # 128 rows per core
NPAIR = R // 2           # 64 pairs per core
SET = 32                 # pairs per PSUM e-bank accumulation set
ALPHA = 0.2
ESHIFT = 4096.0          # relu-safe positive shift on scores (softmax-inv.)
LN_EPS = 1e-5

# Per-pair static schedule, balancing PE / DVE / ACT (the Pool engine
# supports no general tensor ops on TRN2, and only DVE/ACT can read PSUM):
#  - ZFORM pairs use relu(pre) = max(t+u, -V) + V: one DVE TT-max plus an
#    extra matmul M2b + c_j row -> shifts elementwise work onto PE.
#  - Y pairs: DVE TT-add (+V), then lrelu via y=0.2*pre + max(pre,y).
# All PSUM extraction (p = psum + u) runs on ACT.
ZFORM = [(_k % 4) != 2 for _k in range(NPAIR)]               # 48 Z-pairs
# y-op engine for Y pairs: ACT for half of them
Y_ON_ACT = [False for _k in range(NPAIR)]

_CACHE = {}


def _build_program():
    nc = bacc.Bacc("TRN2", target_bir_lowering=False, debug=False,
                   num_devices=NCORES)

    def din(name, shape, dt):
        return nc.dram_tensor(name, shape, dt, kind="ExternalInput").ap()

    ejT2 = din("ejT2", [2 * D, N], BF16)     # [ej^T; ej^T]
    V2 = din("V2", [2 * D, N], BF16)         # [V; V]
    nV2 = din("nV2", [2 * D, N], BF16)       # [-V; -V]
    cB = din("cB", [R, N], BF16)              # c_j rows (zeroed for Y-pairs)
    lhsT1 = din("lhsT1", [2 * D, 2 * D], BF16)
    lhsT2a = din("lhsT2a", [2 * D, SET * D], BF16)
    lhsT2b = din("lhsT2b", [2 * D, SET * D], BF16)
    iden = din("iden", [128, 128], F32)
    Whbf = din("Whbf", [128, 8 * D], BF16)   # 8 chunks [128, 64]
    eib2 = din("eib2", [2 * D, NPAIR], F32)  # col k = [ei_{2k}+b; ei_{2k+1}+b]
    u2 = din("u2", [2 * D, NPAIR], F32)      # col k = [u_{2k}; u_{2k+1}]
    adjf = din("adjf", [R, N], BF16)
    hrows = din("hrows", [R, D], F32)
    lngr = din("lngr", [R, D], F32)
    lnbr = din("lnbr", [R, D], F32)
    out_d = nc.dram_tensor("out", [R, D], F32, kind="ExternalOutput").ap()

    with tile.TileContext(nc) as tc, \
         tc.tile_pool(name="static", bufs=1) as sp:
        # ---------------- static SBUF tiles ----------------
        ejT2_sb = sp.tile([2 * D, N], BF16, name="ejT2_sb", tag="t_ejT2")
        V2_sb = sp.tile([2 * D, N], BF16, name="V2_sb", tag="t_V2")
        nV2_sb = sp.tile([2 * D, N], BF16, name="nV2_sb", tag="t_nV2")
        cB_sb = sp.tile([R, N], BF16, name="cB_sb", tag="t_cB")
        lhsT1_sb = sp.tile([2 * D, 2 * D], BF16, name="lhsT1_sb", tag="t_l1")
        lhsT2a_sb = sp.tile([2 * D, SET * D], BF16, name="lhsT2a_sb", tag="t_l2a")
        lhsT2b_sb = sp.tile([2 * D, SET * D], BF16, name="lhsT2b_sb", tag="t_l2b")
        Wh_sb = sp.tile([128, 8 * D], BF16, name="Wh_sb", tag="t_Wh")
        iden_sb = sp.tile([128, 128], F32, name="iden_sb", tag="t_iden")
        eib2_sb = sp.tile([2 * D, NPAIR], F32, name="eib2_sb", tag="t_eib2")
        u2_sb = sp.tile([2 * D, NPAIR], F32, name="u2_sb", tag="t_u2")
        adjf_sb = sp.tile([R, N], BF16, name="adjf_sb", tag="t_adjf")
        hrows_sb = sp.tile([R, D], F32, name="hrows_sb", tag="t_hrows")
        lngr_sb = sp.tile([R, D], F32, name="lngr_sb", tag="t_lngr")
        lnbr_sb = sp.tile([R, D], F32, name="lnbr_sb", tag="t_lnbr")

        NBUF = 4
        rhs1_sb = [sp.tile([2 * D, N], BF16, name=f"rhs1_{b}", tag=f"t_r1{b}")
                   for b in range(NBUF)]
        p_sb = [sp.tile([2 * D, N], BF16, name=f"p_{b}", tag=f"t_p{b}")
                for b in range(NBUF)]
        pre_sb = [sp.tile([2 * D, N], BF16, name=f"pre_{b}", tag=f"t_pre{b}")
                  for b in range(NBUF)]
        y_sb = [sp.tile([2 * D, N], BF16, name=f"y_{b}", tag=f"t_y{b}")
                for b in range(NBUF)]
        e_sb = sp.tile([R, N], F32, name="e_sb", tag="t_e")
        em_sb = sp.tile([R, N], F32, name="em_sb", tag="t_em")
        ex_sb = sp.tile([R, N], F32, name="ex_sb", tag="t_ex")
        exT_sb = sp.tile([128, N], BF16, name="exT_sb", tag="t_exT")
        red_sb = sp.tile([R, 8], F32, name="red_sb", tag="t_red")
        scr_sb = sp.tile([1, 8], F32, name="scr_sb", tag="t_scr")
        hp_sb = sp.tile([R, D], F32, name="hp_sb", tag="t_hp")
        xm_sb = sp.tile([R, D], F32, name="xm_sb", tag="t_xm")
        sq_sb = sp.tile([R, D], F32, name="sq_sb", tag="t_sq")
        o_sb = sp.tile([R, D], F32, name="o_sb", tag="t_o")

        # ---------------- load inputs ----------------
        nc.sync.dma_start(ejT2_sb[:], ejT2)
        nc.sync.dma_start(eib2_sb[:], eib2)
        nc.sync.dma_start(lhsT1_sb[:], lhsT1)
        nc.sync.dma_start(u2_sb[:], u2)
        nc.gpsimd.dma_start(V2_sb[:], V2)
        nc.gpsimd.dma_start(nV2_sb[:], nV2)
        nc.sync.dma_start(lhsT2a_sb[:], lhsT2a)
        nc.sync.dma_start(lhsT2b_sb[:], lhsT2b)
        nc.gpsimd.dma_start(cB_sb[:], cB)
        nc.gpsimd.dma_start(adjf_sb[:], adjf)
        nc.sync.dma_start(Wh_sb[:], Whbf)
        nc.gpsimd.dma_start(iden_sb[:], iden)
        nc.gpsimd.dma_start(hrows_sb[:], hrows)
        nc.gpsimd.dma_start(lngr_sb[:], lngr)
        nc.gpsimd.dma_start(lnbr_sb[:], lnbr)

        # warm ACT table sets early (Exp used at softmax)
        nc.vector.memset(scr_sb[:], 1.0)
        nc.scalar.activation(scr_sb[0:1, 0:1], scr_sb[0:1, 1:2], AF.Exp)
        # warm the PE p-state during the DMA lead-in: the tensor engine's
        # clock ramps to full speed only after ~3us of sustained work
        warm_sb = sp.tile([128, 512], BF16, name="warm_sb", tag="t_warm")
        nc.vector.memset(warm_sb[:], 0.0)
        with tc.tile_pool(name="ps_warm", bufs=1, space="PSUM") as pw:
            wps = pw.tile([128, 512], F32, name="wps")
            for _w in range(2):
                nc.tensor.matmul(wps[:], warm_sb[:, 0:128], warm_sb[:])

        # ---------------- main loop over 64 row-pairs ----------------
        # Software-pipelined with LAG between the produce stage (s1/M1/
        # extract/TT -> pre) and the consume stage (M2 accumulation): M2(k)
        # is emitted after M1(k+LAG) so the in-order PE queue never stalls
        # on pair k's DVE/ACT chain.
        LAG = 2
        with tc.tile_pool(name="ps_mm1", bufs=3, space="PSUM") as pmm1, \
             tc.tile_pool(name="ps_e", bufs=2, space="PSUM") as pe:
            bankE = None

            def stage_A(k):
                buf = k % NBUF
                zf = ZFORM[k]
                # s1: relu_s pair = relu(ejT2 + (ei + b_e))  [DVE 4x bf16]
                nc.vector.tensor_scalar(
                    rhs1_sb[buf][:], ejT2_sb[:], eib2_sb[:, k:k + 1], 0.0,
                    op0=ALU.add, op1=ALU.max)
                # M1: psum = 0.8 A_e^T relu_s (paired, 128-contract)
                psum = pmm1.tile([2 * D, N], F32, name="psum", tag="psum")
                for jh in range(2):
                    nc.tensor.matmul(
                        psum[:, jh * 512:(jh + 1) * 512], lhsT1_sb[:],
                        rhs1_sb[buf][:, jh * 512:(jh + 1) * 512])
                # PSUM extract on ACT: p = psum + u  (bf16 out)
                nc.scalar.activation(p_sb[buf][:], psum[:], AF.Identity,
                                     bias=u2_sb[:, k:k + 1], scale=1.0)
                if zf:
                    # Z-form: z' = max(p, -V)
                    nc.vector.tensor_tensor(pre_sb[buf][:], p_sb[buf][:],
                                            nV2_sb[:], op=ALU.max)
                else:
                    # Y-form: pre = p + V, then lrelu = max(pre, 0.2*pre)
                    nc.vector.tensor_tensor(pre_sb[buf][:], p_sb[buf][:],
                                            V2_sb[:], op=ALU.add)
                    if Y_ON_ACT[k]:
                        nc.scalar.activation(y_sb[buf][:], pre_sb[buf][:],
                                             AF.Copy, bias=0.0, scale=ALPHA)
                    else:
                        nc.vector.tensor_scalar(y_sb[buf][:], pre_sb[buf][:],
                                                ALPHA, None, op0=ALU.mult)
                    nc.vector.tensor_tensor(
                        pre_sb[buf][:], pre_sb[buf][:], y_sb[buf][:],
                        op=ALU.max)

            def stage_B(k):
                nonlocal bankE
                g = k % SET
                buf = k % NBUF
                zf = ZFORM[k]
                if g == 0:
                    bankE = [pe.tile([R // 2, 512], F32, name="bankE",
                                     tag="bankE") for _ in range(2)]
                # M2a: accumulate pair scores into e-banks
                for jh in range(2):
                    nc.tensor.matmul(
                        bankE[jh][:], lhsT2a_sb[:, g * D:(g + 1) * D],
                        pre_sb[buf][:, jh * 512:(jh + 1) * 512],
                        start=(g == 0), stop=(g == SET - 1 and not zf))
                    if zf:
                        nc.tensor.matmul(
                            bankE[jh][:], lhsT2b_sb[:, g * D:(g + 1) * D],
                            rhs1_sb[buf][:, jh * 512:(jh + 1) * 512],
                            start=False, stop=(g == SET - 1))
                if g == SET - 1:
                    s = k // SET
                    rows = slice(s * R // 2, (s + 1) * R // 2)
                    # fused copy-out: e = bankE + ESHIFT + c_j (cB rows are
                    # zero for Y-form pairs); PSUM readers: DVE/ACT only
                    for jh in range(2):
                        nc.vector.scalar_tensor_tensor(
                            e_sb[rows, jh * 512:(jh + 1) * 512],
                            bankE[jh][:, :], ESHIFT,
                            cB_sb[rows, jh * 512:(jh + 1) * 512],
                            op0=ALU.add, op1=ALU.add)

            for k in range(NPAIR + LAG):
                if k < NPAIR:
                    stage_A(k)
                if k >= LAG:
                    stage_B(k - LAG)

        # ---------------- mask + softmax ----------------
        nc.vector.tensor_tensor(em_sb[:], e_sb[:], adjf_sb[:], op=ALU.mult)
        nc.vector.reduce_max(red_sb[:, 0:1], em_sb[:], axis=AX.X)
        nc.vector.tensor_scalar(red_sb[:, 1:2], red_sb[:, 0:1], -1.0, None,
                                op0=ALU.mult)
        nc.scalar.activation(ex_sb[:], em_sb[:], AF.Exp,
                             bias=red_sb[:, 1:2], scale=1.0,
                             accum_out=red_sb[:, 2:3])
        nc.vector.reciprocal(red_sb[:, 3:4], red_sb[:, 2:3])

        # ---------------- h' = softmax @ Wh + h ; LayerNorm ----------------
        with tc.tile_pool(name="ps_fin", bufs=4, space="PSUM") as pf:
            for t in range(8):
                tp_ps = pf.tile([128, 128], F32, name="tp_ps", tag="tp")
                nc.tensor.transpose(tp_ps[:], ex_sb[:, t * 128:(t + 1) * 128],
                                    iden_sb[:])
                nc.scalar.copy(exT_sb[:, t * 128:(t + 1) * 128], tp_ps[:])
            hp_ps = pf.tile([R, D], F32, name="hp_ps", bufs=1)
            for t in range(8):
                nc.tensor.matmul(hp_ps[:], exT_sb[:, t * 128:(t + 1) * 128],
                                 Wh_sb[:, t * D:(t + 1) * D],
                                 start=(t == 0), stop=(t == 7))
            # h' = F * recip + h
            nc.vector.scalar_tensor_tensor(
                hp_sb[:], hp_ps[:], red_sb[:, 3:4], hrows_sb[:],
                op0=ALU.mult, op1=ALU.add)

        nc.vector.reduce_sum(red_sb[:, 4:5], hp_sb[:], axis=AX.X)
        nc.vector.tensor_scalar(red_sb[:, 5:6], red_sb[:, 4:5], 1.0 / D, None,
                                op0=ALU.mult)
        nc.vector.tensor_scalar(xm_sb[:], hp_sb[:], red_sb[:, 5:6], None,
                                op0=ALU.subtract)
        nc.vector.tensor_tensor(sq_sb[:], xm_sb[:], xm_sb[:], op=ALU.mult)
        nc.vector.reduce_sum(red_sb[:, 6:7], sq_sb[:], axis=AX.X)
        # rstd = 1/sqrt(var + eps)
        nc.vector.tensor_scalar(red_sb[:, 6:7], red_sb[:, 6:7], 1.0 / D,
                                LN_EPS, op0=ALU.mult, op1=ALU.add)
        nc.scalar.activation(red_sb[:, 7:8], red_sb[:, 6:7], AF.Sqrt)
        nc.vector.reciprocal(red_sb[:, 7:8], red_sb[:, 7:8])
        nc.vector.tensor_scalar(xm_sb[:], xm_sb[:], red_sb[:, 7:8], None,
                                op0=ALU.mult)
        nc.vector.tensor_tensor(o_sb[:], xm_sb[:], lngr_sb[:], op=ALU.mult)
        nc.vector.tensor_tensor(o_sb[:], o_sb[:], lnbr_sb[:], op=ALU.add)
        nc.sync.dma_start(out_d, o_sb[:])

    nc.compile()
    return nc


def _host_prep(inputs):
    h = np.asarray(inputs["h"], np.float32)[0]            # [N, D]
    adj = np.asarray(inputs["adj"])[0]                    # [N, N] int32
    W = np.asarray(inputs["W"], np.float32)
    attn_w1 = np.asarray(inputs["attn_w1"], np.float32)
    attn_b1 = np.asarray(inputs["attn_b1"], np.float32)
    attn_w2 = np.asarray(inputs["attn_w2"], np.float32)
    edge_w = np.asarray(inputs["edge_w"], np.float32)
    edge_b = np.asarray(inputs["edge_b"], np.float32)
    ln_g = np.asarray(inputs["ln_g"], np.float32)
    ln_b = np.asarray(inputs["ln_b"], np.float32)

    A_i, A_j, A_e = attn_w1[:D], attn_w1[D:2 * D], attn_w1[2 * D:]
    E_i, E_j = edge_w[:D], edge_w[D:]
    w2 = attn_w2[:, 0]

    ej = h @ E_j                                          # [N, D]
    eib = h @ E_i + edge_b                                # [N, D]
    Mv = W @ A_j + ALPHA * (E_j @ A_e)
    V = (h @ Mv).T                                        # [D, N]
    u = h @ (W @ A_i) + ALPHA * (eib @ A_e) + attn_b1     # [N, D]
    cj = (h @ Mv) @ w2                                    # [N]
    Wh = h @ W                                            # [N, D]

    ejT2 = np.vstack([ej.T, ej.T])
    V2 = np.vstack([V, V])

    lhsT1 = np.zeros((2 * D, 2 * D), np.float32)
    lhsT1[:D, :D] = 0.8 * A_e
    lhsT1[D:, D:] = 0.8 * A_e

    lhsT2a = np.zeros((2 * D, SET * D), np.float32)
    lhsT2b = np.zeros((2 * D, SET * D), np.float32)
    g1 = ALPHA * 0.8 * (A_e @ w2)                         # 0.16 A_e w2
    for g in range(SET):
        # block g serves pairs k=g and k=g+SET; ZFORM has period 4 so both
        # pairs agree on the weight scaling.
        wa = 0.8 * w2 if ZFORM[g] else w2
        lhsT2a[:D, g * D + 2 * g] = wa
        lhsT2a[D:, g * D + 2 * g + 1] = wa
        lhsT2b[:D, g * D + 2 * g] = g1
        lhsT2b[D:, g * D + 2 * g + 1] = g1

    Whpack = np.zeros((128, 8 * D), np.float32)
    for t in range(8):
        Whpack[:, t * D:(t + 1) * D] = Wh[t * 128:(t + 1) * 128]

    bf = ml_dtypes.bfloat16
    rep = {
        "ejT2": ejT2.astype(bf),
        "V2": V2.astype(bf),
        "nV2": (-V2).astype(bf),
        "lhsT1": lhsT1.astype(bf),
        "lhsT2a": lhsT2a.astype(bf),
        "lhsT2b": lhsT2b.astype(bf),
        "Whbf": Whpack.astype(bf),
        "iden": np.eye(128, dtype=np.float32),
    }
    in_maps = []
    for c in range(NCORES):
        rows = slice(c * R, (c + 1) * R)
        m = dict(rep)
        # paired per-core column tensors: col k = [row 2k ; row 2k+1]
        eib_c = eib[rows]                                 # [128, 64]
        u_c = u[rows]
        eib2 = np.empty((2 * D, NPAIR), np.float32)
        u2 = np.empty((2 * D, NPAIR), np.float32)
        for k in range(NPAIR):
            eib2[:D, k] = eib_c[2 * k]
            eib2[D:, k] = eib_c[2 * k + 1]
            u2[:D, k] = u_c[2 * k]
            u2[D:, k] = u_c[2 * k + 1]
        cB = np.zeros((R, N), np.float32)
        for k in range(NPAIR):
            if ZFORM[k]:
                s, g = k // SET, k % SET
                cB[s * 64 + 2 * g] = cj
                cB[s * 64 + 2 * g + 1] = cj
        m.update({
            "eib2": eib2,
            "u2": u2,
            "cB": cB.astype(bf),
            "adjf": adj[rows].astype(np.float32).astype(bf),
            "hrows": np.ascontiguousarray(h[rows]),
            "lngr": np.broadcast_to(ln_g, (R, D)).copy(),
            "lnbr": np.broadcast_to(ln_b, (R, D)).copy(),
        })
        in_maps.append(m)
    return in_maps


def _get_nc():
    if "nc" not in _CACHE:
        _CACHE["nc"] = _build_program()
    return _CACHE["nc"]


def kernel(**inputs) -> np.ndarray:
    nc = _get_nc()
    in_maps = _host_prep(inputs)
    res = run_bass_kernel_spmd(nc, in_maps, list(range(NCORES))).results
    out = np.concatenate([res[c]["out"] for c in range(NCORES)], axis=0)
    return out[None].astype(np.float32)
